# revision 1
# baseline (speedup 1.0000x reference)
"""LocallyConnected2d Bass kernel for 8 TRN2 NeuronCores.

Problem: out[b,o,oh,ow] = sum_{c,kh,kw} x[b,c,oh+kh-1,ow+kw-1] * w[o,c,oh,ow,kh*3+kw]
Shapes: x (8,64,32,32) f32, weight (1,64,64,32,32,9) f32 -> out (8,64,32,32) f32.

Sharding: each core owns 4 consecutive output rows (oh). The 144 MiB weight
tensor is the dominant traffic; this split reads it exactly once (18.9 MB/core)
with no duplication and needs no collectives.

Per-core kernel: every output location is an independent tiny matmul
  out_loc[b, o] = patches_loc[ck, b].T @ w_loc[ck, o]
PSUM-accumulated over tap groups (M=b=8, N=o=64). The 9 taps are packed
into 5 matmuls per location ("tap pairing"): the 128 contraction partitions
hold (tapA c | tapB c), where partitions 64-127 of the x tile hold a
pre-shifted copy of the input so one access-pattern base offset addresses
both taps. Pairs (0,1)(3,4)(6,7) use a (0,+1)-column-shifted copy, pair
(2,5) a (+1,0)-row-shifted copy, and the leftover tap 8 of adjacent even/odd
columns shares one 128-partition group (K=64 matmuls on each half).

Weights are pre-arranged on the host into exactly this partition layout,
chunk-contiguous, so every weight DMA is one fully contiguous
128-partition transfer.
"""

import numpy as np
import ml_dtypes

import concourse.bacc as bacc
import concourse.bass as bass
import concourse.tile as tile
from concourse import mybir
from concourse.bass_utils import run_bass_kernel_spmd

B, C, O = 8, 64, 64
OH, OW = 32, 32
NCORES = 8
R = OH // NCORES          # 4 oh rows per core
HS = R + 2                # x halo rows per core
WS = OW + 2               # padded width
F32 = mybir.dt.float32

# Tap pairing: slots 0-3 are (tapA, tapB) pairs; taps are k = 3*kh + kw.
PAIRS = [(0, 1), (3, 4), (6, 7), (2, 5)]
# lhsT base (kh, kw, which x tile) per pair slot; x tile 0 = column-shifted
# duplicate in partitions 64+, tile 1 = row-shifted duplicate.
PAIR_BASE = [(0, 0, 0), (1, 0, 0), (2, 0, 0), (0, 2, 1)]

USE_BF16 = True
DT = mybir.dt.bfloat16 if USE_BF16 else F32
NPDT = ml_dtypes.bfloat16 if USE_BF16 else np.float32

_cache: dict = {}
_last_in_maps = None


def _build() -> bass.Bass:
    nc = bacc.Bacc("TRN2", target_bir_lowering=False, debug=False,
                   num_devices=NCORES)
    # x patches: [0:64] = slab [c,b,h,w]; [64:128] = shifted duplicate.
    xa = nc.dram_tensor("xa", [128, B, HS, WS], DT, kind="ExternalInput").ap()
    xb = nc.dram_tensor("xb", [128, B, HS, WS], DT, kind="ExternalInput").ap()
    # weights: [oh_l, p, slot, o, owp] chunk-contiguous.
    ws = nc.dram_tensor("ws", [R, 128, 9, O, OW // 2], DT,
                        kind="ExternalInput").ap()
    out = nc.dram_tensor("out", [B, R, OW, O], F32, kind="ExternalOutput").ap()

    with tile.TileContext(nc) as tc:
        with (
            tc.tile_pool(name="xpool", bufs=1) as xpool,
            tc.tile_pool(name="wpool", bufs=2) as wpool,
            tc.tile_pool(name="opool", bufs=1) as opool,
            tc.tile_pool(name="pspool", bufs=6, space="PSUM") as pspool,
        ):
            x_sb = [xpool.tile([128, B, HS, WS], DT, name="xa_sb"),
                    xpool.tile([128, B, HS, WS], DT, name="xb_sb")]
            nc.sync.dma_start(x_sb[0][:], xa)
            nc.sync.dma_start(x_sb[1][:], xb)

            out_sb = opool.tile([B, R, OW, O], F32)

            for oh_l in range(R):
                w_sb = wpool.tile([128, 9, O, OW // 2], DT, tag="wt")
                nc.sync.dma_start(w_sb[:], ws[oh_l])
                pt = None
                for owp in range(OW // 2):
                    for eo in range(2):
                        ow = 2 * owp + eo
                        if ow % 8 == 0:
                            pt = pspool.tile([B, 8, O], F32, tag="ps",
                                             name=f"ps_{oh_l}_{ow // 8}")
                        po = pt[:, ow % 8, :]
                        for s in range(4):
                            kh, kw, xt = PAIR_BASE[s]
                            nc.tensor.matmul(
                                po,
                                x_sb[xt][:, :, oh_l + kh, ow + kw],
                                w_sb[:, 4 * eo + s, :, owp],
                                start=(s == 0), stop=False)
                        if eo == 0:  # tap 8 via unshifted half
                            lhsT = x_sb[0][0:64, :, oh_l + 2, ow + 2]
                            rhs = w_sb[0:64, 8, :, owp]
                        else:        # tap 8 via column-shifted half
                            lhsT = x_sb[0][64:128, :, oh_l + 2, ow + 1]
                            rhs = w_sb[64:128, 8, :, owp]
                        nc.tensor.matmul(po, lhsT, rhs, start=False, stop=True)
                        if ow % 8 == 7:
                            nc.vector.tensor_copy(
                                out=out_sb[:, oh_l, ow - 7:ow + 1, :],
                                in_=pt[:])

            nc.sync.dma_start(out, out_sb[:])
    nc.compile()
    return nc


def _marshal(x: np.ndarray, weight: np.ndarray) -> list[dict]:
    x = np.ascontiguousarray(x, dtype=np.float32)
    w = weight[0]  # (O, C, OH, OW, K)

    # Padded input (B, C, OH+2, OW+2); core r reads padded rows [R*r, R*r+HS)
    xp = np.zeros((B, C, OH + 2, OW + 2), dtype=np.float32)
    xp[:, :, 1:OH + 1, 1:OW + 1] = x

    in_maps = []
    for r in range(NCORES):
        slab = xp[:, :, R * r:R * r + HS, :].transpose(1, 0, 2, 3)  # c,b,h,w
        sw = np.zeros_like(slab)
        sw[:, :, :, :WS - 1] = slab[:, :, :, 1:]        # column shift
        sh = np.zeros_like(slab)
        sh[:, :, :HS - 1, :] = slab[:, :, 1:, :]        # row shift
        xa_r = np.concatenate([slab, sw], axis=0).astype(NPDT)
        xb_r = np.concatenate([slab, sh], axis=0).astype(NPDT)

        # weight slab -> [oh_l, p, slot, o, owp]
        wt = w[:, :, R * r:R * (r + 1), :, :].transpose(2, 1, 0, 3, 4)
        # wt: [oh, c, o, ow, k]
        even, odd = wt[:, :, :, 0::2, :], wt[:, :, :, 1::2, :]
        W2 = np.empty((R, 128, 9, O, OW // 2), dtype=np.float32)
        for s, (ka, kb) in enumerate(PAIRS):
            W2[:, 0:64, s] = even[..., ka]
            W2[:, 64:128, s] = even[..., kb]
            W2[:, 0:64, 4 + s] = odd[..., ka]
            W2[:, 64:128, 4 + s] = odd[..., kb]
        W2[:, 0:64, 8] = even[..., 8]
        W2[:, 64:128, 8] = odd[..., 8]
        in_maps.append({
            "xa": np.ascontiguousarray(xa_r),
            "xb": np.ascontiguousarray(xb_r),
            "ws": np.ascontiguousarray(W2.astype(NPDT)),
        })
    return in_maps


def kernel(x: np.ndarray, weight: np.ndarray) -> np.ndarray:
    global _last_in_maps
    in_maps = _marshal(x, weight)
    _last_in_maps = in_maps

    if "nc" not in _cache:
        _cache["nc"] = _build()
    res = run_bass_kernel_spmd(_cache["nc"], in_maps, list(range(NCORES)))

    # Per-core out is (B, R, OW, O); stitch to (B, O, OH, OW).
    parts = [res.results[r]["out"].transpose(0, 3, 1, 2) for r in range(NCORES)]
    return np.ascontiguousarray(np.concatenate(parts, axis=2))



# revision 2
# speedup vs baseline: 1.6016x; 1.6016x over previous
"""LocallyConnected2d Bass kernel for 8 TRN2 NeuronCores.

Problem: out[b,o,oh,ow] = sum_{c,kh,kw} x[b,c,oh+kh-1,ow+kw-1] * w[o,c,oh,ow,kh*3+kw]
Shapes: x (8,64,32,32) f32, weight (1,64,64,32,32,9) f32 -> out (8,64,32,32) f32.

Sharding: each core owns 4 consecutive output rows (oh). The 144 MiB weight
tensor is the dominant traffic; this split reads it exactly once (9.4 MB/core
in bf16) with no duplication and needs no collectives.

Per-core kernel: every output location is an independent tiny matmul
  out_loc[b, o] = patches_loc[ck, b].T @ w_loc[ck, o]
PSUM-accumulated over tap groups (M=b=8, N=o=64). The 9 taps are packed
into 5 matmuls per location ("tap pairing"): the 128 contraction partitions
hold (tapA c | tapB c), where partitions 64-127 of the x tile hold a
pre-shifted copy of the input so one access-pattern base offset addresses
both taps. Pairs (0,1)(3,4)(6,7) use a (0,+1)-column-shifted copy, pair
(2,5) a (+1,0)-row-shifted copy, and the leftover tap 8 of adjacent even/odd
columns shares one 128-partition group (K=64 matmuls on each half).

Both matmul operands are laid out so their streamed dimension is contiguous
per partition (x tiles carry B innermost, weight tiles carry O innermost) —
strided PE operand reads cost ~8x (16B SBUF line per 2B element). Weights
are pre-arranged on the host into exactly this layout, chunk-contiguous per
(oh row, half-row), and all 8 chunk DMAs are issued up front so the 16 SDMA
engines stream at full HBM rate while the PE consumes chunk k.
"""

import numpy as np
import ml_dtypes

import concourse.bacc as bacc
import concourse.bass as bass
import concourse.tile as tile
from concourse import mybir
from concourse.bass_utils import run_bass_kernel_spmd

B, C, O = 8, 64, 64
OH, OW = 32, 32
NCORES = 8
R = OH // NCORES          # 4 oh rows per core
HS = R + 2                # x halo rows per core
WS = OW + 2               # padded width
F32 = mybir.dt.float32
NCHUNK = 2 * R            # weight DMA chunks per core (half an oh row each)

# Tap pairing: slots 0-3 are (tapA, tapB) pairs; taps are k = 3*kh + kw.
PAIRS = [(0, 1), (3, 4), (6, 7), (2, 5)]
# lhsT base (kh, kw, which x tile) per pair slot; x tile 0 = column-shifted
# duplicate in partitions 64+, tile 1 = row-shifted duplicate.
PAIR_BASE = [(0, 0, 0), (1, 0, 0), (2, 0, 0), (0, 2, 1)]

USE_BF16 = True
DT = mybir.dt.bfloat16 if USE_BF16 else F32
NPDT = ml_dtypes.bfloat16 if USE_BF16 else np.float32

_cache: dict = {}
_last_in_maps = None


def _build() -> bass.Bass:
    nc = bacc.Bacc("TRN2", target_bir_lowering=False, debug=False,
                   num_devices=NCORES)
    # x patches, B innermost (contiguous lhsT): [0:64] = slab [c,h,w,b];
    # [64:128] = shifted duplicate.
    xa = nc.dram_tensor("xa", [128, HS, WS, B], DT, kind="ExternalInput").ap()
    xb = nc.dram_tensor("xb", [128, HS, WS, B], DT, kind="ExternalInput").ap()
    # weights: [oh_l, blk, p, owp_in_blk, slot, o], contiguous per (oh_l, blk)
    # chunk and O innermost (contiguous rhs).
    ws = nc.dram_tensor("ws", [R, 2, 128, 8, 9, O], DT,
                        kind="ExternalInput").ap()
    out = nc.dram_tensor("out", [B, R, OW, O], F32, kind="ExternalOutput").ap()

    with tile.TileContext(nc) as tc:
        with (
            tc.tile_pool(name="xpool", bufs=1) as xpool,
            tc.tile_pool(name="wpool", bufs=NCHUNK) as wpool,
            tc.tile_pool(name="opool", bufs=1) as opool,
            tc.tile_pool(name="pspool", bufs=6, space="PSUM") as pspool,
        ):
            x_sb = [xpool.tile([128, HS, WS, B], DT, name="xa_sb"),
                    xpool.tile([128, HS, WS, B], DT, name="xb_sb")]
            nc.sync.dma_start(x_sb[0][:], xa)
            nc.sync.dma_start(x_sb[1][:], xb)

            # Prefetch every weight chunk immediately; all NCHUNK buffers
            # stay resident so the DMA queue drains back-to-back.
            w_sb = []
            for ci in range(NCHUNK):
                wt = wpool.tile([128, 8, 9, O], DT, tag="wt", name=f"w_{ci}")
                nc.sync.dma_start(wt[:], ws[ci // 2, ci % 2])
                w_sb.append(wt)

            out_sb = opool.tile([B, R, OW, O], F32)

            for oh_l in range(R):
                for blk in range(2):
                    wt = w_sb[2 * oh_l + blk]
                    for j in range(8):
                        for eo in range(2):
                            ow = 16 * blk + 2 * j + eo
                            if ow % 8 == 0:
                                pt = pspool.tile([B, 8, O], F32, tag="ps",
                                                 name=f"ps_{oh_l}_{ow // 8}")
                            po = pt[:, ow % 8, :]
                            for s in range(4):
                                kh, kw, xt = PAIR_BASE[s]
                                nc.tensor.matmul(
                                    po,
                                    x_sb[xt][:, oh_l + kh, ow + kw, :],
                                    wt[:, j, 4 * eo + s, :],
                                    start=(s == 0), stop=False)
                            if eo == 0:  # tap 8 via unshifted half
                                lhsT = x_sb[0][0:64, oh_l + 2, ow + 2, :]
                                rhs = wt[0:64, j, 8, :]
                            else:        # tap 8 via column-shifted half
                                lhsT = x_sb[0][64:128, oh_l + 2, ow + 1, :]
                                rhs = wt[64:128, j, 8, :]
                            nc.tensor.matmul(po, lhsT, rhs,
                                             start=False, stop=True)
                            if ow % 8 == 7:
                                nc.vector.tensor_copy(
                                    out=out_sb[:, oh_l, ow - 7:ow + 1, :],
                                    in_=pt[:])
                nc.sync.dma_start(out[:, oh_l], out_sb[:, oh_l])
    nc.compile()
    return nc


def _marshal(x: np.ndarray, weight: np.ndarray) -> list[dict]:
    x = np.ascontiguousarray(x, dtype=np.float32)
    w = weight[0]  # (O, C, OH, OW, K)

    # Padded input (B, C, OH+2, OW+2); core r reads padded rows [R*r, R*r+HS)
    xp = np.zeros((B, C, OH + 2, OW + 2), dtype=np.float32)
    xp[:, :, 1:OH + 1, 1:OW + 1] = x

    in_maps = []
    for r in range(NCORES):
        # [c, h, w, b] with b innermost so lhsT reads are contiguous
        slab = xp[:, :, R * r:R * r + HS, :].transpose(1, 2, 3, 0)
        sw = np.zeros_like(slab)
        sw[:, :, :WS - 1, :] = slab[:, :, 1:, :]        # column shift
        sh = np.zeros_like(slab)
        sh[:, :HS - 1, :, :] = slab[:, 1:, :, :]        # row shift
        xa_r = np.concatenate([slab, sw], axis=0).astype(NPDT)
        xb_r = np.concatenate([slab, sh], axis=0).astype(NPDT)

        # weight slab -> [oh_l, p, slot, o, owp]
        wt = w[:, :, R * r:R * (r + 1), :, :].transpose(2, 1, 0, 3, 4)
        # wt: [oh, c, o, ow, k]
        even, odd = wt[:, :, :, 0::2, :], wt[:, :, :, 1::2, :]
        W2 = np.empty((R, 128, 9, O, OW // 2), dtype=np.float32)
        for s, (ka, kb) in enumerate(PAIRS):
            W2[:, 0:64, s] = even[..., ka]
            W2[:, 64:128, s] = even[..., kb]
            W2[:, 0:64, 4 + s] = odd[..., ka]
            W2[:, 64:128, 4 + s] = odd[..., kb]
        W2[:, 0:64, 8] = even[..., 8]
        W2[:, 64:128, 8] = odd[..., 8]
        # -> [oh, owp, p, slot, o] -> chunked [oh, blk, p, owp_in_blk, slot, o]
        W3 = W2.transpose(0, 4, 1, 2, 3).reshape(R, 2, 8, 128, 9, O)
        W3 = W3.transpose(0, 1, 3, 2, 4, 5)
        in_maps.append({
            "xa": np.ascontiguousarray(xa_r),
            "xb": np.ascontiguousarray(xb_r),
            "ws": np.ascontiguousarray(W3.astype(NPDT)),
        })
    return in_maps


def kernel(x: np.ndarray, weight: np.ndarray) -> np.ndarray:
    global _last_in_maps
    in_maps = _marshal(x, weight)
    _last_in_maps = in_maps

    if "nc" not in _cache:
        _cache["nc"] = _build()
    res = run_bass_kernel_spmd(_cache["nc"], in_maps, list(range(NCORES)))

    # Per-core out is (B, R, OW, O); stitch to (B, O, OH, OW).
    parts = [res.results[r]["out"].transpose(0, 3, 1, 2) for r in range(NCORES)]
    return np.ascontiguousarray(np.concatenate(parts, axis=2))
